# revision 15
# baseline (speedup 1.0000x reference)
"""Trainium2 Bass kernel for MHA cross-attention (nn_MHACross).

Sharding: 8 cores = 2 batches x 4 head-groups (2 heads each).
Per core (batch b, head group g): q = x[b] @ Wq[g].T ; k,v = xmel[b] @
Wkv[g].T ; RoPE on q,k (scale folded into host cos/sin tables, bf16);
per head scores^T = k_r @ q_r^T (bf16); p = exp(scores - 2.5) stored
fp8e4 in st-pair-interleaved layout; unnormalized out2 = v^T @ p and
Z = ones^T @ p run as fp8 DoubleRow matmuls (K=256 per pass), with Z
additionally 4-way column-tiled (M=1 streams in separate PE column
groups) and split into two PSUM accumulators so the 1/Z chain starts a
group early; ao = out2 * (1/Z) (1/Z broadcast across partitions by
gpsimd) stored fp8 head-interleaved; y_partial = ao @ Wout[:, g].T as
fp8 DoubleRow (both heads in one pass).  Host sums the 4 partial y per
batch.  The constant -2.5 exp offset cancels between numerator and
denominator; Wout is scaled x16 on host (fp8 subnormal avoidance) and
divided back at the y evacuation.

Schedule: inputs are host-permuted into contiguous 1MB chunks streamed
across all three DMA rings (sync + scalar HWDGE, gpsimd SWDGE) in
strict need-order; attention on head 0 starts ~15us in, and remaining
projections are emitted as prework interleaved into the attention
pipeline.  RoPE half-swap runs as partition-offset DVE multiplies with
the final add on gpsimd.
"""
import sys
sys.path.insert(0, '/opt/trn_rl_repo')
import numpy as np

DIM = 1024
NHEADS = 8
HD = 128          # head dim
HPC = 2           # heads per core
NG = 4            # head groups (cores per batch)
B, T, S = 2, 2048, 3000
NKT = DIM // 128  # contraction tiles
ROPE_BASE = 10000.0
CW = 512          # chunk width (t and s)
PAIR = 2 * CW     # paired t-chunk width for exp
NTC = T // CW     # 4 t-chunks
NSC = (S + CW - 1) // CW   # 6 s-chunks
SPAD = NSC * CW   # 3072 padded S
N_ST = (S + 127) // 128    # 24 s-tiles of 128
NPP = N_ST // 2   # 12 st-pairs per block
G = 4             # s-tiles per pipeline group
NGRP = N_ST // G  # 6 groups per block
DELAY = 2
EXPOFF = 2.5      # exp(s - EXPOFF): keeps p in fp8e4 range; cancels in softmax

# fallback flags
import os
FP8_AV = os.environ.get("FP8_AV", "0") == "1"    # p/v in fp8, DoubleRow attn@V
Z_DR = os.environ.get("Z_DR", "0") == "1"        # DoubleRow Z (else plain fp8 Z)
FP8_OUT = os.environ.get("FP8_OUT", "0") == "1"  # fp8 DoubleRow out-projection
ROPE_SWAP = "vector"   # vector | scalar

_cache = {}


def build_nc():
    from concourse import bacc, mybir
    from concourse.tile import TileContext

    f32 = mybir.dt.float32
    bf16 = mybir.dt.bfloat16
    f8 = mybir.dt.float8e4
    AF = mybir.ActivationFunctionType
    DR = mybir.MatmulPerfMode.DoubleRow

    p_dt = f8 if FP8_AV else bf16
    exp_bias = -EXPOFF if FP8_AV else 0.0

    nc = bacc.Bacc("TRN2", target_bir_lowering=False, debug=False,
                   num_devices=8, num_swdge_queues=2)

    xQ = nc.dram_tensor("xQ", [NTC, 128, NKT, CW], bf16, kind="ExternalInput")
    xmQ = nc.dram_tensor("xmQ", [NSC, 128, NKT, CW], bf16, kind="ExternalInput")
    WqT = nc.dram_tensor("WqT", [128, NKT * HPC * HD], bf16, kind="ExternalInput")
    WkT = nc.dram_tensor("WkT", [128, NKT * HPC * HD], bf16, kind="ExternalInput")
    WvT = nc.dram_tensor("WvT", [128, NKT * HPC * HD], bf16, kind="ExternalInput")
    if FP8_OUT:
        WoP = nc.dram_tensor("WoP", [128, HPC, DIM], f8, kind="ExternalInput")
    else:
        WoT = nc.dram_tensor("WoT", [HPC * HD, DIM], bf16, kind="ExternalInput")
    cosq = nc.dram_tensor("cosq", [128, T], bf16, kind="ExternalInput")
    sinq = nc.dram_tensor("sinq", [128, T], bf16, kind="ExternalInput")
    cosk = nc.dram_tensor("cosk", [128, S], bf16, kind="ExternalInput")
    sink = nc.dram_tensor("sink", [128, S], bf16, kind="ExternalInput")
    y = nc.dram_tensor("y", [T, DIM], bf16, kind="ExternalOutput")

    s_chunks = [(i * CW, min(CW, S - i * CW)) for i in range(NSC)]

    with TileContext(nc) as tc:
        with tc.tile_pool(name="wpool", bufs=1) as wp:
            # ---- persistent tiles ----
            wq = wp.tile([128, NKT, HPC * HD], bf16)
            wk = wp.tile([128, NKT, HPC * HD], bf16)
            wv = wp.tile([128, NKT, HPC * HD], bf16)
            if FP8_OUT:
                wo_pair = wp.tile([128, HPC, DIM], f8)
            else:
                wo = [wp.tile([128, DIM], bf16, name=f"wo{h}", uniquify=True)
                      for h in range(HPC)]
            if FP8_AV:
                ones3p = wp.tile([128, 2, 16], f8)
                nc.vector.memset(ones3p[:], 1.0)
                ones3 = ones3p[:, :, 0:1]
                ebias = wp.tile([128, 1], f32)
                nc.vector.memset(ebias[:], -EXPOFF)
            else:
                ones = wp.tile([128, 1], bf16)
                nc.vector.memset(ones[:], 1.0)

            qT_r = [wp.tile([128, T], bf16, name=f"qT{h}", uniquify=True)
                    for h in range(HPC)]
            kT_r = [wp.tile([128, S], bf16, name=f"kT{h}", uniquify=True)
                    for h in range(HPC)]
            v_sb = wp.tile([128, N_ST, HPC * HD], p_dt)

            csq = [(wp.tile([128, CW], bf16, name=f"cq{c}", uniquify=True),
                    wp.tile([128, CW], bf16, name=f"sq{c}", uniquify=True))
                   for c in range(NTC)]
            csk = [(wp.tile([128, CW], bf16, name=f"ck{c}", uniquify=True),
                    wp.tile([128, CW], bf16, name=f"sk{c}", uniquify=True))
                   for c in range(NSC)]

            xm = [wp.tile([128, NKT, CW], bf16, name=f"xm{c}", uniquify=True)
                  for c in range(NSC)]

            xq = [None] * NTC

            with tc.tile_pool(name="xqp", bufs=1) as xqp, \
                 tc.tile_pool(name="rtP", bufs=2) as rtp, \
                 tc.tile_pool(name="pP", bufs=10) as pP, \
                 tc.tile_pool(name="aoP", bufs=4) as aoP, \
                 tc.tile_pool(name="zrP", bufs=2) as zrP, \
                 tc.tile_pool(name="zbP", bufs=2) as zbP, \
                 tc.tile_pool(name="yP", bufs=2) as yP, \
                 tc.tile_pool(name="psA", bufs=2, space="PSUM") as psA:

                for c in range(NTC):
                    xq[c] = xqp.tile([128, NKT, CW], bf16, name=f"xq{c}",
                                     uniquify=True, tag="xq", bufs=NTC)

                # ---- DMA prologue, strict need-order per ring ----
                # gpsimd SWDGE ring (starts earliest, ~138GB/s)
                nc.gpsimd.dma_start(out=wq[:], in_=WqT[:].rearrange("p (k n) -> p k n", k=NKT))
                nc.gpsimd.dma_start(out=xm[1][:], in_=xmQ[1])
                nc.gpsimd.dma_start(out=xm[4][:], in_=xmQ[4])
                nc.gpsimd.dma_start(out=xm[5][:], in_=xmQ[5])
                if FP8_OUT:
                    nc.gpsimd.dma_start(out=wo_pair[:], in_=WoP[:])
                else:
                    for h in range(HPC):
                        nc.gpsimd.dma_start(out=wo[h][:], in_=WoT[h * HD:(h + 1) * HD, :])

                # sync HWDGE ring
                nc.sync.dma_start(out=xq[0][:], in_=xQ[0])
                nc.sync.dma_start(out=xm[0][:], in_=xmQ[0])
                nc.sync.dma_start(out=xq[2][:], in_=xQ[2])
                nc.sync.dma_start(out=xm[3][:], in_=xmQ[3])

                # scalar HWDGE ring
                def dma_cs(dst_pair, cos_d, sin_d, c0, cw):
                    nc.scalar.dma_start(out=dst_pair[0][:, :cw], in_=cos_d[:, c0:c0 + cw])
                    nc.scalar.dma_start(out=dst_pair[1][:, :cw], in_=sin_d[:, c0:c0 + cw])

                dma_cs(csq[0], cosq, sinq, 0, CW)
                nc.scalar.dma_start(out=xq[1][:], in_=xQ[1])
                nc.scalar.dma_start(out=wk[:], in_=WkT[:].rearrange("p (k n) -> p k n", k=NKT))
                dma_cs(csk[0], cosk, sink, 0, CW)
                nc.scalar.dma_start(out=wv[:], in_=WvT[:].rearrange("p (k n) -> p k n", k=NKT))
                nc.scalar.dma_start(out=xm[2][:], in_=xmQ[2])
                dma_cs(csk[1], cosk, sink, CW, CW)
                dma_cs(csq[1], cosq, sinq, CW, CW)
                nc.scalar.dma_start(out=xq[3][:], in_=xQ[3])
                dma_cs(csk[2], cosk, sink, 2 * CW, CW)
                dma_cs(csq[2], cosq, sinq, 2 * CW, CW)
                dma_cs(csq[3], cosq, sinq, 3 * CW, CW)
                dma_cs(csk[3], cosk, sink, 3 * CW, CW)
                dma_cs(csk[4], cosk, sink, 4 * CW, CW)
                dma_cs(csk[5], cosk, sink, 5 * CW, S - 5 * CW)

                # ---- projection + RoPE ----
                def rope_from_ps(ps, cos_sb, sin_sb, out_sl, cw):
                    swp = rtp.tile([128, CW], f32, name="swp", tag="rt", bufs=2)
                    if ROPE_SWAP == "vector":
                        nc.vector.tensor_mul(swp[0:64, :cw], ps[64:128, :cw], sin_sb[0:64, :cw])
                        nc.vector.tensor_mul(swp[64:128, :cw], ps[0:64, :cw], sin_sb[64:128, :cw])
                    else:
                        nc.scalar.copy(swp[0:64, :cw], ps[64:128, :cw])
                        nc.scalar.copy(swp[64:128, :cw], ps[0:64, :cw])
                        nc.vector.tensor_mul(swp[:, :cw], swp[:, :cw], sin_sb[:, :cw])
                    nc.vector.tensor_mul(out_sl, ps[:, :cw], cos_sb[:, :cw])
                    nc.gpsimd.tensor_add(out_sl, out_sl, swp[:, :cw])

                def proj_rope(h, w_sb, src, cs, c0, cw, out_r):
                    ps = psA.tile([128, PAIR], f32, name="prps", tag="sc", bufs=2)
                    for kt in range(NKT):
                        nc.tensor.matmul(
                            ps[:, :cw],
                            w_sb[:, kt, h * HD:(h + 1) * HD],
                            src[:, kt, :cw],
                            start=(kt == 0), stop=(kt == NKT - 1))
                    rope_from_ps(ps, cs[0], cs[1], out_r[:, c0:c0 + cw], cw)

                def q_chunk(c):
                    for h in range(HPC):
                        proj_rope(h, wq, xq[c], csq[c], c * CW, CW, qT_r[h])

                def k_chunk(h, c):
                    c0, cw = s_chunks[c]
                    proj_rope(h, wk, xm[c], csk[c], c0, cw, kT_r[h])

                def v_chunk(c):
                    for j in range(G):
                        st = G * c + j
                        s0 = st * 128
                        scnt = min(128, S - s0)
                        vps = psA.tile([128, HPC * HD], f32, name="vps", tag="sc", bufs=2)
                        for kt in range(NKT):
                            nc.tensor.matmul(
                                vps[:scnt, :],
                                xm[c][:, kt, j * 128:j * 128 + scnt],
                                wv[:, kt, :],
                                start=(kt == 0), stop=(kt == NKT - 1))
                        if scnt < 128:
                            nc.vector.memset(v_sb[:, st, :], 0.0)
                        nc.vector.tensor_copy(v_sb[:scnt, st, :], vps[:scnt, :])

                # ---- attention pipeline ----
                BLOCKS = [(0, 0), (0, 1), (1, 0), (1, 1)]
                blocks = {}
                ao_pairs = {}  # (pi, ci) -> fp8 head-interleaved ao tile

                def sc_exp(bi, g):
                    h, pi = BLOCKS[bi]
                    bk = blocks.setdefault(bi, {"pt": {}})
                    for j in range(G):
                        st = G * g + j
                        s0 = st * 128
                        scnt = min(128, S - s0)
                        scps = psA.tile([128, PAIR], f32, name="scps", tag="sc", bufs=2)
                        for ci in range(2):
                            c0 = pi * PAIR + ci * CW
                            nc.tensor.matmul(
                                scps[:scnt, ci * CW:(ci + 1) * CW],
                                kT_r[h][:, s0:s0 + scnt],
                                qT_r[h][:, c0:c0 + CW],
                                start=True, stop=True,
                                skip_group_check=True)
                        if FP8_AV:
                            pp, ko = st // 2, st % 2
                            if ko == 0:
                                p_t = pP.tile([128, 2, PAIR], f8, name="p_t", tag="p", bufs=10)
                                bk["pt"][pp] = p_t
                                if pp == NPP - 1:
                                    # zero the odd half first (st 23 writes only 56 rows)
                                    nc.vector.memset(p_t[:, 1, :], 0.0)
                            else:
                                p_t = bk["pt"][pp]
                            nc.scalar.activation(p_t[:scnt, ko, :], scps[:scnt, :],
                                                 AF.Exp, bias=ebias[:scnt, 0:1])
                        else:
                            p_t = pP.tile([128, PAIR], bf16, name="p_t", tag="p", bufs=10)
                            nc.scalar.activation(p_t[:scnt, :], scps[:scnt, :], AF.Exp)
                            bk["pt"][st] = (p_t, scnt)

                def z_finalize_a(bk):
                    # partial Z = sum of the two A-strips, per ci (starts a group early)
                    bk["zt"] = []
                    for ci in range(2):
                        zt = zrP.tile([1, CW], f32, name="zt", tag="zr", bufs=2)
                        nc.vector.tensor_copy(zt[0:1, :], bk["zpsA"][64 * ci:64 * ci + 1, :])
                        nc.vector.tensor_add(zt[0:1, :], zt[0:1, :],
                                             bk["zpsA"][64 * ci + 32:64 * ci + 33, :])
                        bk["zt"].append(zt)

                def z_finalize_b(bk, h, pi):
                    bk["zr2"] = []
                    for ci in range(2):
                        zt = bk["zt"][ci]
                        nc.vector.tensor_add(zt[0:1, :], zt[0:1, :],
                                             bk["zpsB"][64 * ci:64 * ci + 1, :])
                        nc.vector.tensor_add(zt[0:1, :], zt[0:1, :],
                                             bk["zpsB"][64 * ci + 32:64 * ci + 33, :])
                        zrec = zrP.tile([1, CW], f32, name="zrec", tag="zc", bufs=2)
                        nc.vector.reciprocal_approx_fast(out=zrec[0:1, :], in_=zt[0:1, :])
                        zr2 = zbP.tile([128, CW], f32, name="zr2", tag="zb", bufs=2)
                        nc.gpsimd.partition_broadcast(zr2[:, :], zrec[0:1, :])
                        bk["zr2"].append(zr2)

                def zav(bi, g):
                    h, pi = BLOCKS[bi]
                    bk = blocks[bi]
                    last = (g == NGRP - 1)
                    if g == 0:
                        bk["zpsA"] = psA.tile([128, CW], f32, name="zpsA", tag="z", bufs=2)
                        bk["o2"] = [psA.tile([128, CW], f32, name="o2ps", tag="acc", bufs=2)
                                    for _ in range(2)]
                    if last:
                        bk["zpsB"] = psA.tile([128, CW], f32, name="zpsB", tag="z", bufs=2)

                    if FP8_AV:
                        pps = [G * g // 2 + jj for jj in range(G // 2)]
                        # Z: 4 concurrent M=1 streams (2 pp x 2 ci), DoubleRow optional
                        for pp in pps:
                            for ci in range(2):
                                strip = 64 * ci + 32 * (pp % 2)
                                zdst = bk["zpsB"] if last else bk["zpsA"]
                                st_, sp_ = ((True, True) if last
                                            else (pp < 2, pp >= NPP - 4))
                                if Z_DR:
                                    nc.tensor.matmul(
                                        zdst[strip:strip + 1, :CW],
                                        ones3,
                                        bk["pt"][pp][:, :, ci * CW:(ci + 1) * CW],
                                        start=st_, stop=sp_,
                                        perf_mode=DR,
                                        tile_position=(0, strip),
                                        skip_group_check=True)
                                else:
                                    for ko in range(2):
                                        nc.tensor.matmul(
                                            zdst[strip:strip + 1, :CW],
                                            ones3p[:, ko, 0:1],
                                            bk["pt"][pp][:, ko, ci * CW:(ci + 1) * CW],
                                            start=(st_ and ko == 0),
                                            stop=(sp_ and ko == 1),
                                            tile_position=(0, strip),
                                            skip_group_check=True)
                        if g == NGRP - 2:
                            z_finalize_a(bk)
                        if last:
                            z_finalize_b(bk, h, pi)
                        # attn @ V: DoubleRow over st-pairs
                        for pp in pps:
                            for ci in range(2):
                                nc.tensor.matmul(
                                    bk["o2"][ci][:, :CW],
                                    v_sb[:, 2 * pp:2 * pp + 2, h * HD:(h + 1) * HD],
                                    bk["pt"][pp][:, :, ci * CW:(ci + 1) * CW],
                                    start=(pp == 0), stop=(pp == NPP - 1),
                                    perf_mode=DR,
                                    skip_group_check=True)
                    else:
                        for j in range(G):
                            st = G * g + j
                            p_t, scnt = bk["pt"][st]
                            for ci in range(2):
                                strip = 64 * ci + 32 * (st % 2)
                                zdst = bk["zpsB"] if last else bk["zpsA"]
                                st_, sp_ = ((st < N_ST - 2, st >= N_ST - 2) if last
                                            else (st < 2, st >= N_ST - 6))
                                nc.tensor.matmul(
                                    zdst[strip:strip + 1, :CW],
                                    ones[:scnt, :1],
                                    p_t[:scnt, ci * CW:(ci + 1) * CW],
                                    start=st_, stop=sp_,
                                    tile_position=(0, strip),
                                    skip_group_check=True)
                        if g == NGRP - 2:
                            z_finalize_a(bk)
                        if last:
                            z_finalize_b(bk, h, pi)
                        for j in range(G):
                            st = G * g + j
                            p_t, scnt = bk["pt"][st]
                            for ci in range(2):
                                nc.tensor.matmul(
                                    bk["o2"][ci][:, :CW],
                                    v_sb[:scnt, st, h * HD:(h + 1) * HD],
                                    p_t[:scnt, ci * CW:(ci + 1) * CW],
                                    start=(st == 0), stop=(st == N_ST - 1),
                                    skip_group_check=True)
                    if last:
                        bk["ao"] = []
                        for ci in range(2):
                            if FP8_OUT:
                                key = (pi, ci)
                                if key not in ao_pairs:
                                    ao_pairs[key] = aoP.tile([128, 2, CW], f8, name="aop",
                                                             tag="ao", bufs=4)
                                nc.vector.tensor_mul(ao_pairs[key][:, h, :],
                                                     bk["o2"][ci][:, :], bk["zr2"][ci][:, :])
                            else:
                                ao_t = aoP.tile([128, CW], bf16, name="ao", tag="aob", bufs=6)
                                nc.vector.tensor_mul(ao_t[:, :], bk["o2"][ci][:, :],
                                                     bk["zr2"][ci][:, :])
                                bk["ao"].append(ao_t)

                def outproj(pi):
                    for tt in range(PAIR // 128):
                        ci, tl = tt // 4, (tt % 4) * 128
                        y_sb = yP.tile([128, DIM], bf16, name="y_sb", tag="ysb", bufs=2)
                        for nn in range(2):
                            yps = psA.tile([128, CW], f32, name="yps", tag="sc", bufs=2)
                            if FP8_OUT:
                                nc.tensor.matmul(
                                    yps[:, :],
                                    ao_pairs[(pi, ci)][:, :, tl:tl + 128],
                                    wo_pair[:, :, nn * CW:(nn + 1) * CW],
                                    start=True, stop=True,
                                    perf_mode=DR,
                                    skip_group_check=True)
                            else:
                                b0 = blocks[BLOCKS.index((0, pi))]
                                b1 = blocks[BLOCKS.index((1, pi))]
                                for hh, bk in enumerate((b0, b1)):
                                    nc.tensor.matmul(
                                        yps[:, :],
                                        bk["ao"][ci][:, tl:tl + 128],
                                        wo[hh][:, nn * CW:(nn + 1) * CW],
                                        start=(hh == 0), stop=(hh == 1),
                                        skip_group_check=True)
                            ysl = y_sb[:, nn * CW:(nn + 1) * CW]
                            scl = (1.0 / 16.0) if FP8_OUT else 1.0
                            if pi == 1:
                                nc.scalar.mul(ysl, yps[:, :], scl)
                            else:
                                nc.vector.tensor_scalar_mul(ysl, yps[:, :], scl)
                        r0 = pi * PAIR + tt * 128
                        nc.sync.dma_start(out=y[r0:r0 + 128, :], in_=y_sb[:, :])

                # prologue: q pair0, first k chunk
                q_chunk(0)
                q_chunk(1)
                k_chunk(0, 0)

                prework = {}
                for g in range(NGRP):
                    w = []
                    if g == 2:
                        w.append(lambda: q_chunk(2))
                    if g == 3:
                        w.append(lambda: q_chunk(3))
                    w.append(lambda c=g: v_chunk(c))
                    if g + 1 < NSC:
                        w.append(lambda c=g + 1: k_chunk(0, c))
                    prework[(0, g)] = w
                for g in range(NGRP):
                    prework[(1, g)] = [lambda c=g: k_chunk(1, c)]

                groups = [(bi, g) for bi in range(4) for g in range(NGRP)]

                def finish(key):
                    bi, g = key
                    zav(bi, g)
                    # outproj(0) deferred 2 groups past block2 so the PE isn't
                    # stalled on the 1/Z -> ao chain; outproj(1) at the end
                    if (bi, g) == (3, 1):
                        outproj(0)
                    if (bi, g) == (3, NGRP - 1):
                        outproj(1)

                for i, key in enumerate(groups):
                    for w in prework.get(key, ()):
                        w()
                    sc_exp(*key)
                    if i >= DELAY:
                        finish(groups[i - DELAY])
                for j in range(max(0, len(groups) - DELAY), len(groups)):
                    finish(groups[j])

    nc.compile()
    return nc


def _host_tables():
    scale = float(HD) ** (-0.25)
    inv = 1.0 / (ROPE_BASE ** (np.arange(0, HD, 2, dtype=np.float64) / HD))  # [64]

    def tables(L):
        fr = np.outer(inv, np.arange(L, dtype=np.float64))  # [64, L]
        c = np.cos(fr) * scale
        s = np.sin(fr) * scale
        cos = np.concatenate([c, c], axis=0)
        sin = np.concatenate([-s, s], axis=0)
        return cos, sin

    return tables(T), tables(S)


def make_in_maps(x, xmel, Wq, Wkv, Wout):
    import ml_dtypes
    bf = ml_dtypes.bfloat16
    f8 = ml_dtypes.float8_e4m3fn
    (cosq_, sinq_), (cosk_, sink_) = _host_tables()
    cosq_, sinq_ = cosq_.astype(bf), sinq_.astype(bf)
    cosk_, sink_ = cosk_.astype(bf), sink_.astype(bf)

    x = np.asarray(x, dtype=np.float32)
    xmel = np.asarray(xmel, dtype=np.float32)
    Wq = np.asarray(Wq, dtype=np.float32)
    Wkv = np.asarray(Wkv, dtype=np.float32)
    Wout = np.asarray(Wout, dtype=np.float32)

    # x[b]: [T, DIM] -> [NTC, 128, NKT, CW] with xQ[c,p,k,t] = x[c*CW+t, k*128+p]
    xQ_b = [np.ascontiguousarray(
        x[b].reshape(NTC, CW, NKT, 128).transpose(0, 3, 2, 1)).astype(bf)
        for b in range(B)]
    xmp = np.zeros((B, SPAD, DIM), dtype=np.float32)
    xmp[:, :S, :] = xmel
    xmQ_b = [np.ascontiguousarray(
        xmp[b].reshape(NSC, CW, NKT, 128).transpose(0, 3, 2, 1)).astype(bf)
        for b in range(B)]

    gsz = HPC * HD  # 256
    WqT_g, WkT_g, WvT_g, Wo_g = [], [], [], []
    for g in range(NG):
        r0 = g * gsz

        def prearr(wt):  # [DIM, gsz] -> [128, NKT*gsz], row p holds [kt, n]
            return np.ascontiguousarray(
                wt.reshape(NKT, 128, gsz).transpose(1, 0, 2).reshape(128, NKT * gsz)).astype(bf)

        WqT_g.append(prearr(Wq[r0:r0 + gsz, :].T))
        WkT_g.append(prearr(Wkv[r0:r0 + gsz, :].T))
        WvT_g.append(prearr(Wkv[DIM + r0:DIM + r0 + gsz, :].T))
        wog = Wout[:, r0:r0 + gsz].T  # [256, DIM]
        if FP8_OUT:
            # [256, DIM] -> [128, 2, DIM] head-interleaved, x16 (fp8 subnormals)
            wop = np.ascontiguousarray(
                (wog.reshape(HPC, HD, DIM).transpose(1, 0, 2) * 16.0)).astype(f8)
            Wo_g.append(wop)
        else:
            Wo_g.append(np.ascontiguousarray(wog).astype(bf))

    in_maps = []
    for c in range(B * NG):
        b, g = c // NG, c % NG
        m = {
            "xQ": xQ_b[b], "xmQ": xmQ_b[b],
            "WqT": WqT_g[g], "WkT": WkT_g[g], "WvT": WvT_g[g],
            "cosq": cosq_, "sinq": sinq_, "cosk": cosk_, "sink": sink_,
        }
        if FP8_OUT:
            m["WoP"] = Wo_g[g]
        else:
            m["WoT"] = Wo_g[g]
        in_maps.append(m)
    return in_maps


def kernel(x, xmel, Wq, Wkv, Wout):
    from concourse.bass_utils import run_bass_kernel_spmd

    x = np.asarray(x, dtype=np.float32)
    xmel = np.asarray(xmel, dtype=np.float32)
    Bx, Tx, C = x.shape
    Sx = xmel.shape[1]
    assert (Bx, Tx, C, Sx) == (B, T, DIM, S)

    if "nc" not in _cache:
        _cache["nc"] = build_nc()
    nc = _cache["nc"]

    in_maps = make_in_maps(x, xmel,
                           np.asarray(Wq, dtype=np.float32),
                           np.asarray(Wkv, dtype=np.float32),
                           np.asarray(Wout, dtype=np.float32))
    res = run_bass_kernel_spmd(nc, in_maps, list(range(8)))
    out = np.zeros((B, T, DIM), dtype=np.float32)
    for c in range(8):
        b = c // NG
        out[b] += res.results[c]["y"].astype(np.float32)
    return out


# revision 16
# speedup vs baseline: 1.0485x; 1.0485x over previous
"""Trainium2 Bass kernel for MHA cross-attention (nn_MHACross).

Sharding: 8 cores = 2 batches x 4 head-groups (2 heads each).
Each core computes, for its (batch b, head group g):
    q = x[b] @ Wq[g].T ; k,v = xmel[b] @ Wkv[g].T ; RoPE(q, k) (scale folded
    into host-side cos/sin tables); per head scores^T = k_r @ q_r^T;
    p = exp(scores) with no max subtraction (scores are O(6) here, safe in
    fp32); unnormalized out2 = v^T @ p and Z = ones^T @ p on the PE;
    normalize by 1/Z; y_partial = attn @ Wout[:, g].T.  Host sums the 4
    partial y per batch.

Layouts keep the contraction dim on partitions everywhere; no on-device
transposes.  Matmul operands are bf16 (except the final projection, which
runs in float32r); PSUM accumulation is fp32 throughout.  The attention
inner loop is batched by op type (all scores, then all attnV, then all Z
matmuls per head/chunk-pair) so the PE streams back-to-back with stationary
reuse, while exp for both 512-chunks of a pair runs as one [128,1024]
scalar-engine instruction.
"""
import sys
sys.path.insert(0, '/opt/trn_rl_repo')
import numpy as np

DIM = 1024
NHEADS = 8
HD = 128          # head dim
HPC = 2           # heads per core
NG = 4            # head groups (cores per batch)
B, T, S = 2, 2048, 3000
NKT = DIM // 128  # contraction tiles
ROPE_BASE = 10000.0
CW = 512          # T-chunk width
PAIR = 2 * CW     # paired chunk width for exp

_cache = {}


def _ceil_div(a, b):
    return (a + b - 1) // b


def build_nc(T=T, S=S):
    from concourse import bacc, mybir
    from concourse.tile import TileContext

    f32 = mybir.dt.float32
    f32r = mybir.dt.float32r
    bf16 = mybir.dt.bfloat16

    nc = bacc.Bacc("TRN2", target_bir_lowering=False, debug=False, num_devices=8)

    xT = nc.dram_tensor("xT", [DIM, T], bf16, kind="ExternalInput")
    xmelT = nc.dram_tensor("xmelT", [DIM, S], bf16, kind="ExternalInput")
    WqT = nc.dram_tensor("WqT", [128, NKT * HPC * HD], bf16, kind="ExternalInput")
    WkT = nc.dram_tensor("WkT", [128, NKT * HPC * HD], bf16, kind="ExternalInput")
    WvT = nc.dram_tensor("WvT", [128, NKT * HPC * HD], bf16, kind="ExternalInput")
    WoT = nc.dram_tensor("WoT", [HPC * HD, DIM], bf16, kind="ExternalInput")
    cosq = nc.dram_tensor("cosq", [HD, T], f32, kind="ExternalInput")
    sinq = nc.dram_tensor("sinq", [HD, T], f32, kind="ExternalInput")
    cosk = nc.dram_tensor("cosk", [HD, S], f32, kind="ExternalInput")
    sink = nc.dram_tensor("sink", [HD, S], f32, kind="ExternalInput")
    y = nc.dram_tensor("y", [T, DIM], f32, kind="ExternalOutput")
    # DRAM scratch for the Z-broadcast bounce (internal DRAM tensors fail to
    # load under the axon PJRT path, so declare it as an output)
    n_tc = _ceil_div(T, CW)
    zsd = nc.dram_tensor("zs", [HPC * n_tc, CW], f32, kind="ExternalOutput")

    n_st = _ceil_div(S, 128)
    s_chunks = [(i * 512, min(512, S - i * 512)) for i in range(_ceil_div(S, 512))]
    t_chunks = [(i * CW, min(CW, T - i * CW)) for i in range(n_tc)]
    t_pairs = [t_chunks[i:i + 2] for i in range(0, n_tc, 2)]

    with TileContext(nc) as tc:
        with tc.tile_pool(name="wpool", bufs=1) as wp, \
             tc.tile_pool(name="persist", bufs=1) as pp:
            # persistent weights
            wq = wp.tile([128, NKT, HPC * HD], bf16)
            wk = wp.tile([128, NKT, HPC * HD], bf16)
            wv = wp.tile([128, NKT, HPC * HD], bf16)
            wo = []
            for h in range(HPC):
                wo_h = wp.tile([128, DIM], bf16, name=f"wo{h}", uniquify=True)
                nc.gpsimd.dma_start(out=wo_h[:], in_=WoT[h * HD:(h + 1) * HD, :])
                wo.append(wo_h)
            ones = wp.tile([128, 1], bf16)
            nc.vector.memset(ones[:], 1.0)

            # persistent activations
            kT_r = [pp.tile([128, S], bf16, name=f"kT{h}", uniquify=True) for h in range(HPC)]
            qT_r = [pp.tile([128, T], bf16, name=f"qT{h}", uniquify=True) for h in range(HPC)]
            v_sb = pp.tile([128, n_st, HPC * HD], bf16)

            # ---------------- unified schedule ----------------
            # PSUM pools are shared between projection and attention via tags
            # so the two can overlap: psA("sc") serves proj psums, score
            # psums, and outproj psums; psB("acc") serves v-proj psums and
            # attnV accumulators; psC("z") the Z accumulators.
            with tc.tile_pool(name="xmelp", bufs=NKT) as xp, \
                 tc.tile_pool(name="csP", bufs=4) as csp, \
                 tc.tile_pool(name="rtP", bufs=3) as rtp, \
                 tc.tile_pool(name="aoP", bufs=2 * HPC + 2) as aoP, \
                 tc.tile_pool(name="zP", bufs=4) as zP, \
                 tc.tile_pool(name="yP", bufs=2) as yP, \
                 tc.tile_pool(name="psA", bufs=2, space="PSUM") as psA, \
                 tc.tile_pool(name="psB", bufs=2, space="PSUM") as psB, \
                 tc.tile_pool(name="psC", bufs=2, space="PSUM") as psC:
                # weight gathers go on the gpsimd DMA queues so they run in
                # parallel with the x/xmel streams on the sync queue
                nc.gpsimd.dma_start(out=wq[:], in_=WqT[:].rearrange("p (k n) -> p k n", k=NKT))
                nc.gpsimd.dma_start(out=wk[:], in_=WkT[:].rearrange("p (k n) -> p k n", k=NKT))
                nc.gpsimd.dma_start(out=wv[:], in_=WvT[:].rearrange("p (k n) -> p k n", k=NKT))

                def proj_rope(h, c0, cw, w_sb, src, cos_sb, sin_sb, out_sl):
                    ps = psA.tile([128, 512], f32, name="prps", tag="sc", bufs=2)
                    for kt in range(NKT):
                        nc.tensor.matmul(
                            ps[:, :cw],
                            w_sb[:, kt, h * HD:(h + 1) * HD],
                            src[kt][:, c0:c0 + cw],
                            start=(kt == 0), stop=(kt == NKT - 1))
                    swp = rtp.tile([128, 512], f32, name="swp", tag="rt", bufs=3)
                    nc.vector.tensor_mul(swp[0:64, :cw], ps[64:128, :cw], sin_sb[0:64, :cw])
                    nc.vector.tensor_mul(swp[64:128, :cw], ps[0:64, :cw], sin_sb[64:128, :cw])
                    nc.vector.tensor_mul(out_sl, ps[:, :cw], cos_sb[:, :cw])
                    nc.vector.tensor_add(out_sl, out_sl, swp[:, :cw])

                def load_cs(cos_d, sin_d, c0, cw):
                    cos_sb = csp.tile([128, 512], f32, name="cos_sb", tag="cos", bufs=2)
                    sin_sb = csp.tile([128, 512], f32, name="sin_sb", tag="sin", bufs=2)
                    nc.gpsimd.dma_start(out=cos_sb[:, :cw], in_=cos_d[:, c0:c0 + cw])
                    nc.gpsimd.dma_start(out=sin_sb[:, :cw], in_=sin_d[:, c0:c0 + cw])
                    return cos_sb, sin_sb

                # q projection first (x is the smaller input); xq space is
                # freed before the attention p-tiles pool opens
                with tc.tile_pool(name="xqp", bufs=NKT) as xqp:
                    xq = []
                    for kt in range(NKT):
                        xq_t = xqp.tile([128, T], bf16, name=f"xq{kt}", uniquify=True, tag="xq", bufs=NKT)
                        nc.sync.dma_start(out=xq_t[:], in_=xT[kt * 128:(kt + 1) * 128, :])
                        xq.append(xq_t)
                    xm = []
                    for kt in range(NKT):
                        xm_t = xp.tile([128, S], bf16, name=f"xm{kt}", uniquify=True, tag="xm", bufs=NKT)
                        nc.sync.dma_start(out=xm_t[:], in_=xmelT[kt * 128:(kt + 1) * 128, :])
                        xm.append(xm_t)
                    for (c0, cw) in t_chunks:
                        cos_sb, sin_sb = load_cs(cosq, sinq, c0, cw)
                        for h in range(HPC):
                            proj_rope(h, c0, cw, wq, xq, cos_sb, sin_sb, qT_r[h][:, c0:c0 + cw])

                with tc.tile_pool(name="pP", bufs=n_st + 4) as pP:
                    # k projection head 0, then v, then head 1 -- so attention
                    # on head 0 can start while head 1 is still projecting
                    for h in range(HPC):
                        for (c0, cw) in s_chunks:
                            cos_sb, sin_sb = load_cs(cosk, sink, c0, cw)
                            proj_rope(h, c0, cw, wk, xm, cos_sb, sin_sb, kT_r[h][:, c0:c0 + cw])
                        if h == 0:
                            for st in range(n_st):
                                s0 = st * 128
                                scnt = min(128, S - s0)
                                vps = psB.tile([128, HPC * HD], f32, name="vps", tag="acc", bufs=2)
                                for kt in range(NKT):
                                    nc.tensor.matmul(
                                        vps[:scnt, :],
                                        xm[kt][:, s0:s0 + scnt],
                                        wv[:, kt, :],
                                        start=(kt == 0), stop=(kt == NKT - 1))
                                nc.vector.tensor_copy(v_sb[:scnt, st, :], vps[:scnt, :])

                    # ---------------- attention + out projection ----------------
                    # Software pipeline at half-block granularity: the PE
                    # stream is [sc half_i+1][z,av half_i] so the in-order PE
                    # never stalls on the scalar engine's exp of the current
                    # tiles.  State lives in `blocks[key]`.
                    G = (n_st + 3) // 4   # pipeline group size (4 groups/block)
                    groups = []
                    for pi in range(len(t_pairs)):
                        for h in range(HPC):
                            for g0 in range(0, n_st, G):
                                groups.append((pi, h, g0, min(G, n_st - g0)))
                    DELAY = 3
                    blocks = {}

                    def emit_sc_exp(key):
                        pi, h, g0, gc = key
                        pair = t_pairs[pi]
                        pw = sum(cw for _, cw in pair)
                        bk = blocks.setdefault((pi, h), {"ptiles": {}})
                        sts = range(g0, g0 + gc)
                        for st in sts:
                            s0 = st * 128
                            scnt = min(128, S - s0)
                            scps = psA.tile([128, PAIR], f32, name="scps", tag="sc", bufs=2)
                            for ci, (c0, cw) in enumerate(pair):
                                nc.tensor.matmul(
                                    scps[:scnt, ci * CW: ci * CW + cw],
                                    kT_r[h][:, s0:s0 + scnt],
                                    qT_r[h][:, c0:c0 + cw],
                                    start=True, stop=True,
                                    skip_group_check=True)
                            p_t = pP.tile([128, PAIR], bf16, name="p_t", tag="p", bufs=n_st + 4)
                            nc.scalar.activation(p_t[:scnt, :pw], scps[:scnt, :pw],
                                                 mybir.ActivationFunctionType.Exp)
                            bk["ptiles"][st] = (p_t, scnt)

                    def emit_zav(key):
                        pi, h, g0, gc = key
                        pair = t_pairs[pi]
                        bk = blocks[(pi, h)]
                        last = (g0 + gc == n_st)
                        if g0 == 0:
                            bk["zps"] = [psC.tile([128, CW], f32, name="zps", tag="z", bufs=2)
                                         for _ in pair]
                            bk["o2"] = [psB.tile([128, CW], f32, name="o2ps", tag="acc", bufs=2)
                                        for _ in pair]
                        sts = list(range(g0, g0 + gc))
                        for st in sts:
                            p_t, scnt = bk["ptiles"][st]
                            for ci, (c0, cw) in enumerate(pair):
                                strip = 64 * ci + 32 * (st % 2)
                                nc.tensor.matmul(
                                    bk["zps"][ci][strip:strip + 1, :cw],
                                    ones[:scnt, :],
                                    p_t[:scnt, ci * CW: ci * CW + cw],
                                    start=(st < 2), stop=(st >= n_st - 2),
                                    tile_position=(0, strip),
                                    skip_group_check=True)
                        if last:
                            # start 1/Z pipeline as soon as Z is complete
                            bk["zr2"] = []
                            for ci, (c0, cw) in enumerate(pair):
                                zrow = h * n_tc + (c0 // CW)
                                zsb = zP.tile([1, CW], f32, name="zsb", tag="zsb", bufs=2)
                                strip = 64 * ci
                                nc.vector.tensor_copy(zsb[:, :cw], bk["zps"][ci][strip:strip + 1, :cw])
                                nc.vector.tensor_add(zsb[:, :cw], zsb[:, :cw],
                                                     bk["zps"][ci][strip + 32:strip + 33, :cw])
                                nc.sync.dma_start(out=zsd[zrow:zrow + 1, :cw], in_=zsb[:, :cw])
                                zrep = zP.tile([128, CW], f32, name="zrep", tag="zrep", bufs=2)
                                nc.sync.dma_start(out=zrep[:, :cw], in_=zsd[zrow, :cw].partition_broadcast(128))
                                zr2 = zP.tile([128, CW], f32, name="zr2", tag="zr2", bufs=2)
                                nc.vector.reciprocal_approx_fast(out=zr2[:, :cw], in_=zrep[:, :cw])
                                bk["zr2"].append(zr2)
                        for st in sts:
                            p_t, scnt = bk["ptiles"][st]
                            for ci, (c0, cw) in enumerate(pair):
                                nc.tensor.matmul(
                                    bk["o2"][ci][:, :cw],
                                    v_sb[:scnt, st, h * HD:(h + 1) * HD],
                                    p_t[:scnt, ci * CW: ci * CW + cw],
                                    start=(st == 0), stop=(st == n_st - 1))
                        if last:
                            bk["ao"] = []
                            for ci, (c0, cw) in enumerate(pair):
                                ao_h = aoP.tile([128, CW], bf16, name="ao", tag="ao", bufs=2 * HPC + 2)
                                nc.vector.tensor_mul(ao_h[:, :cw], bk["o2"][ci][:, :cw], bk["zr2"][ci][:, :cw])
                                bk["ao"].append(ao_h)

                    def emit_outproj(pi):
                        pair = t_pairs[pi]
                        for ci, (c0, cw) in enumerate(pair):
                            for tt in range(cw // 128):
                                y_sb = yP.tile([128, DIM], f32, name="y_sb", tag="ysb", bufs=2)
                                for nn in range(DIM // 512):
                                    yps = psA.tile([128, 512], f32, name="yps", tag="sc", bufs=2)
                                    for h in range(HPC):
                                        nc.tensor.matmul(
                                            yps[:],
                                            blocks[(pi, h)]["ao"][ci][:, tt * 128:(tt + 1) * 128],
                                            wo[h][:, nn * 512:(nn + 1) * 512],
                                            start=(h == 0), stop=(h == HPC - 1))
                                    nc.vector.tensor_copy(y_sb[:, nn * 512:(nn + 1) * 512], yps[:])
                                nc.sync.dma_start(out=y[c0 + tt * 128: c0 + (tt + 1) * 128, :], in_=y_sb[:])

                    def finish(pk):
                        emit_zav(pk)
                        pi, h, g0, gc = pk
                        if g0 + gc == n_st and h == HPC - 1:
                            emit_outproj(pi)

                    for i, key in enumerate(groups):
                        emit_sc_exp(key)
                        if i >= DELAY:
                            finish(groups[i - DELAY])
                    for j in range(max(0, len(groups) - DELAY), len(groups)):
                        finish(groups[j])

    nc.compile()
    return nc


def _host_tables(T=T, S=S):
    scale = float(HD) ** (-0.25)
    inv = 1.0 / (ROPE_BASE ** (np.arange(0, HD, 2, dtype=np.float64) / HD))  # [64]

    def tables(L):
        fr = np.outer(inv, np.arange(L, dtype=np.float64))  # [64, L]
        c = np.cos(fr) * scale
        s = np.sin(fr) * scale
        cos = np.concatenate([c, c], axis=0).astype(np.float32)
        sin = np.concatenate([-s, s], axis=0).astype(np.float32)
        return np.ascontiguousarray(cos), np.ascontiguousarray(sin)

    cosq_, sinq_ = tables(T)
    cosk_, sink_ = tables(S)
    return cosq_, sinq_, cosk_, sink_


def make_in_maps(x, xmel, Wq, Wkv, Wout):
    import ml_dtypes
    bf = ml_dtypes.bfloat16
    Bx, Tx, C = x.shape
    Sx = xmel.shape[1]
    cosq_, sinq_, cosk_, sink_ = _host_tables(Tx, Sx)

    x = np.asarray(x, dtype=np.float32)
    xmel = np.asarray(xmel, dtype=np.float32)
    Wq = np.asarray(Wq, dtype=np.float32)
    Wkv = np.asarray(Wkv, dtype=np.float32)
    Wout = np.asarray(Wout, dtype=np.float32)

    xT_b = [np.ascontiguousarray(x[b].T).astype(bf) for b in range(Bx)]
    xmelT_b = [np.ascontiguousarray(xmel[b].T).astype(bf) for b in range(Bx)]
    gsz = HPC * HD  # 256
    WqT_g, WkT_g, WvT_g, WoT_g = [], [], [], []
    for g in range(NG):
        r0 = g * gsz
        def prearr(wt):  # [DIM, gsz] -> [128, NKT*gsz], row p holds [kt, n]
            return np.ascontiguousarray(
                wt.reshape(NKT, 128, gsz).transpose(1, 0, 2).reshape(128, NKT * gsz)).astype(bf)
        WqT_g.append(prearr(Wq[r0:r0 + gsz, :].T))
        WkT_g.append(prearr(Wkv[r0:r0 + gsz, :].T))
        WvT_g.append(prearr(Wkv[DIM + r0:DIM + r0 + gsz, :].T))
        WoT_g.append(np.ascontiguousarray(Wout[:, r0:r0 + gsz].T).astype(bf))

    in_maps = []
    for c in range(Bx * NG):
        b, g = c // NG, c % NG
        in_maps.append({
            "xT": xT_b[b], "xmelT": xmelT_b[b],
            "WqT": WqT_g[g], "WkT": WkT_g[g], "WvT": WvT_g[g], "WoT": WoT_g[g],
            "cosq": cosq_, "sinq": sinq_, "cosk": cosk_, "sink": sink_,
        })
    return in_maps


def kernel(x, xmel, Wq, Wkv, Wout):
    from concourse.bass_utils import run_bass_kernel_spmd

    x = np.asarray(x, dtype=np.float32)
    xmel = np.asarray(xmel, dtype=np.float32)
    Bx, Tx, C = x.shape
    Sx = xmel.shape[1]
    assert (Bx, Tx, C, Sx) == (B, T, DIM, S)

    if "nc" not in _cache:
        _cache["nc"] = build_nc()
    nc = _cache["nc"]

    in_maps = make_in_maps(x, xmel,
                           np.asarray(Wq, dtype=np.float32),
                           np.asarray(Wkv, dtype=np.float32),
                           np.asarray(Wout, dtype=np.float32))
    res = run_bass_kernel_spmd(nc, in_maps, list(range(8)))
    out = np.zeros((B, T, DIM), dtype=np.float32)
    for c in range(8):
        b = c // NG
        out[b] += res.results[c]["y"]
    return out

